# revision 18
# baseline (speedup 1.0000x reference)
"""Trainium2 Bass kernel for nn_BilateralLayer (guided filter, FFT-conv reference).

The 33x33 Gaussian kernel (theta=R/4, zeroed center tap) is separable:
k2 = outer(g, g) - delta_center (g[R] == 1). Each gf() becomes two 1-D
band-matrix matmuls on the TensorEngine plus a center-tap delta matmul,
with DMA-transposes between the two passes. The per-pixel regularized
3x3 solve runs on the VectorEngine in bf16 (inputs centered by 0.5 on
the host so the covariance cancellation is benign in bf16).

Sharding: 8 cores = 2 batch images x 4 row-bands of 256 rows; each core
gets a 320-row halo-extended slab (halo 2R=32 per side, zero padded).

Self-contained: hardcodes all shapes; host-side prep in numpy.
"""
import sys

if "/opt/trn_rl_repo" not in sys.path:
    sys.path.insert(0, "/opt/trn_rl_repo")

import numpy as np
import ml_dtypes
from contextlib import ExitStack

import concourse.bass as bass
import concourse.tile as tile
from concourse import bacc, mybir
from concourse.bass_utils import run_bass_kernel_spmd

bf16 = ml_dtypes.bfloat16
F32 = mybir.dt.float32
BF16 = mybir.dt.bfloat16
OP = mybir.AluOpType
AF = mybir.ActivationFunctionType

R = 16
EPS = 0.01
B, H, W, C, K = 2, 1024, 1024, 3, 4
N_CORES = 8
ROWS = 256            # output rows per core
EXT = ROWS + 4 * R    # 320
MID = ROWS + 2 * R    # 288
WE = W + 2 * R        # 1056
WV = 1152             # padded V width (9 x 128)
NPL = 25              # moment planes: I(3) p(4) Ip(12) II(6)
NAB = 16              # a(12, c-major) + b(4)

g1 = np.exp(-0.5 * (np.arange(-R, R + 1) ** 2) / (R / 4.0) ** 2).astype(np.float64)


def _band(nk, nm, shift):
    M = np.zeros((nk, nm), np.float64)
    for ki in range(nk):
        for mo in range(nm):
            d = ki - mo + shift
            if 0 <= d <= 2 * R:
                M[ki, mo] = g1[d]
    return M


def _delta(nk, nm, off, sign=-1.0):
    M = np.zeros((nk, nm), np.float64)
    for mo in range(nm):
        if 0 <= mo + off < nk:
            M[mo + off, mo] = sign
    return M


_S1 = float(g1.sum())
_S2D = _S1 * _S1 - 1.0

# bands normalized so conv outputs are gf(x)/S directly (S = S1^2 - 1):
# first pass carries 1/S1, second pass S1/S, center tap 1/S.
BANDS = {
    "B96a": _band(128, 96, 0) / _S1,          # R1 conv_y (first pass)
    "B96b": _band(128, 96, 0) * (_S1 / _S2D),  # R2 conv_y (second pass)
    "B96c2": _band(128, 64, -32) * (_S1 / _S2D),
    "B128": _band(128, 128, 0) * (_S1 / _S2D),  # R1 conv_x (second pass)
    "B32": _band(32, 128, 128) * (_S1 / _S2D),
    "BL": _band(128, 128, -112) / _S1,        # R2 conv_x (first pass)
    "BM": _band(128, 128, R) / _S1,
    "BR": _band(128, 128, 144) / _S1,
    "D128": _delta(128, 128, 0) / _S2D,       # center-tap R1
    "D96": _delta(128, 96, R) / _S2D,         # center-tap R2 chunks 0,1
    "D64": _delta(128, 64, 3 * R) / _S2D,     # center-tap R2 chunk 2
}


def _n_vec():
    v = np.zeros(H, np.float64)
    for y in range(H):
        lo, hi = max(0, y - R), min(H - 1, y + R)
        v[y] = g1[lo - y + R:hi - y + R + 1].sum()
    return v


_NFULL = np.outer(_n_vec(), _n_vec()) - 1.0


# ----------------------------------------------------------------- builder

def _build():
    nc = bacc.Bacc("TRN2", target_bir_lowering=False, debug=False,
                   enable_asserts=False, num_devices=N_CORES)
    nat = nc.dram_tensor("nat", [NPL, EXT, WE], BF16, kind="ExternalInput").ap()
    xT = nc.dram_tensor("xT", [NPL, W, MID], BF16, kind="ExternalInput").ap()
    cy1 = nc.dram_tensor("cy1", [W, 64], F32, kind="ExternalInput").ap()
    cx1a = nc.dram_tensor("cx1a", [32, 224], F32, kind="ExternalInput").ap()
    cx1b = nc.dram_tensor("cx1b", [32, 224], F32, kind="ExternalInput").ap()
    c2x = nc.dram_tensor("c2x", [ROWS, 32], F32, kind="ExternalInput").ap()
    c2y = nc.dram_tensor("c2y", [64, 992], F32, kind="ExternalInput").ap()
    bnd = {k: nc.dram_tensor(k, list(v.shape), BF16, kind="ExternalInput").ap()
           for k, v in BANDS.items()}
    vdr = nc.dram_tensor("vdr", [NPL, MID, WV], BF16, kind="Internal").ap()
    abdr = nc.dram_tensor("abdr", [NAB, W, MID], BF16, kind="Internal").ap()
    cxadr = nc.dram_tensor("cxadr", [NAB, W, MID], BF16, kind="Internal").ap()
    qdr = nc.dram_tensor("qdr", [K, ROWS, W], F32, kind="ExternalOutput").ap()

    with tile.TileContext(nc) as tc, ExitStack() as top:
        cpool = top.enter_context(tc.tile_pool(name="consts", bufs=1))
        Bt = {}
        for k, v in BANDS.items():
            t = cpool.tile(list(v.shape), BF16, tag=f"band_{k}", name=f"band_{k}")
            nc.scalar.dma_start(t[:], bnd[k][:])
            Bt[k] = t

        # ---------------- stage A: conv_y per plane -> vdr ----------------
        with tc.tile_pool(name="stA", bufs=6) as pa, \
             tc.tile_pool(name="psumA", bufs=3, space="PSUM") as psumA:
            for pl in range(NPL):
                vts = []
                for t in range(3):
                    ntile = pa.tile([128, WE], BF16, tag="natin", name="natin")
                    nc.sync.dma_start(ntile[:], nat[pl, 96 * t:96 * t + 128, :])
                    vsb = pa.tile([96, WV], BF16, tag="vout", name="vout")
                    for s in range(3):
                        sl = slice(352 * s, 352 * (s + 1))
                        ps = psumA.tile([96, 352], F32, tag="psA", name="psA")
                        nc.tensor.matmul(ps[:], Bt["B96a"][:], ntile[:, sl],
                                         start=True, stop=True)
                        if (pl + t + s) % 2 == 0:
                            nc.scalar.copy(vsb[:, sl], ps[:])
                        else:
                            nc.vector.tensor_copy(vsb[:, sl], ps[:])
                    nc.vector.memset(vsb[:, WE:WV], 0.0)
                    vts.append(vsb)
                for t in range(3):
                    nc.gpsimd.dma_start(vdr[pl, 96 * t:96 * t + 96, :], vts[t][:])

        # ---------------- stage B: per x-chunk conv_x + solve + R2cx ------
        entries = {}   # chunk -> ab_all tile [128, 16*288]

        with tc.tile_pool(name="stB", bufs=3) as pb, \
             tc.tile_pool(name="stBv", bufs=2) as pv, \
             tc.tile_pool(name="stBc", bufs=2) as pcx, \
             tc.tile_pool(name="stBt", bufs=8) as pt, \
             tc.tile_pool(name="stBA", bufs=2) as pA, \
             tc.tile_pool(name="stB4", bufs=6) as p4, \
             tc.tile_pool(name="stBp", bufs=3) as pk, \
             tc.tile_pool(name="psumB", bufs=6, space="PSUM") as psumB, \
             tc.tile_pool(name="psumB2", bufs=2, space="PSUM") as psumB2:

            def ab_ap(j, p):
                return entries[j][:, 288 * p:288 * (p + 1)]

            def emit_r2cx(j):
                cxa = pcx.tile([128, NAB * MID], BF16, tag="cxa", name="cxa")
                for p in range(NAB):
                    ps = psumB2.tile([128, MID], F32, tag="psB2", name="psB2")
                    nc.tensor.matmul(ps[:], Bt["BM"][:], ab_ap(j, p),
                                     start=True, stop=False)
                    if j > 0:
                        nc.tensor.matmul(ps[:], Bt["BL"][:], ab_ap(j - 1, p),
                                         start=False, stop=(j == 7))
                    if j < 7:
                        nc.tensor.matmul(ps[:], Bt["BR"][:], ab_ap(j + 1, p),
                                         start=False, stop=True)
                    nc.scalar.copy(cxa[:, 288 * p:288 * (p + 1)], ps[:])
                nc.gpsimd.dma_start(
                    cxadr[:, 128 * j:128 * (j + 1), :].rearrange("q p f -> p q f"),
                    cxa[:].rearrange("p (q f) -> p q f", q=NAB))

            vta = {}

            vdr_flat = vdr.rearrange("q m w -> (q m) w")

            def load_vta(ch):
                t = pv.tile([128, NPL * MID], BF16, tag="vta", name="vta")
                nc.sync.dma_start_transpose(
                    t[:], vdr_flat[:, 128 * ch:128 * (ch + 1)])
                vta[ch] = t

            load_vta(0)
            for i in range(8):
                load_vta(i + 1)
                cy = pb.tile([128, 64], F32, tag="cy", name="cy")
                nc.scalar.dma_start(cy[:], cy1[128 * i:128 * (i + 1), :])
                xti = pv.tile([128, NPL * MID], BF16, tag="xti", name="xti")
                nc.scalar.dma_start(
                    xti[:].rearrange("p (q f) -> p q f", q=NPL),
                    xT[:, 128 * i:128 * (i + 1), :].rearrange("q p f -> p q f"))

                mu = [pb.tile([128, MID], BF16, tag=f"mu{c}", name=f"mu{c}")
                      for c in range(3)]
                mup = pb.tile([128, 4 * MID], BF16, tag="mup", name="mup")
                mip = [pb.tile([128, 4 * MID], BF16, tag=f"mip{c}", name=f"mip{c}")
                       for c in range(3)]
                iidx = {(0, 0): 19, (0, 1): 20, (0, 2): 21, (1, 1): 22,
                        (1, 2): 23, (2, 2): 24}
                mii = {cc: pb.tile([128, MID], BF16, tag=f"mii{v}", name=f"mii{v}")
                       for cc, v in iidx.items()}

                def mtarget(pl):
                    if pl < 3:
                        return mu[pl][:]
                    if pl < 7:
                        k = pl - 3
                        return mup[:, 288 * k:288 * (k + 1)]
                    if pl < 19:
                        c, k = divmod(pl - 7, 4)
                        return mip[c][:, 288 * k:288 * (k + 1)]
                    cc = [cc for cc, v in iidx.items() if v == pl][0]
                    return mii[cc][:]

                for pl in range(NPL):
                    psl = slice(288 * pl, 288 * (pl + 1))
                    ps = psumB.tile([128, MID], F32, tag="psB", name="psB")
                    nc.tensor.matmul(ps[:], Bt["B128"][:], vta[i][:, psl],
                                     start=True, stop=False)
                    nc.tensor.matmul(ps[:], Bt["B32"][:],
                                     vta[i + 1][0:32, psl],
                                     start=False, stop=False)
                    nc.tensor.matmul(ps[:], Bt["D128"][:], xti[:, psl],
                                     start=False, stop=True)
                    nc.scalar.copy(mtarget(pl), ps[:])

                # ---- boundary strip fixups: s -> mu = s*c ----
                # y-strips: FD idx [0:32) and [256:288) on all tiles
                for (off, coff) in ((0, 0), (256, 32)):
                    cslc = cy[:, coff:coff + 32]
                    cyb = cy[:, coff:coff + 32] \
                        .rearrange("p (o f) -> p o f", o=1) \
                        .broadcast_to([128, 4, 32])
                    for tile_ in list(mu) + list(mii.values()):
                        nc.vector.tensor_tensor(tile_[:, off:off + 32],
                                                tile_[:, off:off + 32],
                                                cslc, OP.mult)
                    for pk_t in [mup] + mip:
                        sl4 = pk_t[:].rearrange(
                            "p (r f) -> p r f", r=4)[:, :, off:off + 32]
                        nc.vector.tensor_tensor(sl4, sl4, cyb, OP.mult)
                # x-strips: chunk 0 partitions [0:16), chunk 7 [112:128)
                if i in (0, 7):
                    cxs = pb.tile([128, 224], F32, tag="cxs", name="cxs")
                    plo = 0 if i == 0 else 96
                    psl_ = slice(plo, plo + 32)
                    nc.scalar.dma_start(cxs[psl_, :],
                                        (cx1a if i == 0 else cx1b)[:])
                    cxb = cxs[psl_, :].rearrange("p (o f) -> p o f", o=1) \
                                      .broadcast_to([32, 4, 224])
                    for tile_ in list(mu) + list(mii.values()):
                        nc.vector.tensor_tensor(tile_[psl_, 32:256],
                                                tile_[psl_, 32:256],
                                                cxs[psl_, :], OP.mult)
                    for pk_t in [mup] + mip:
                        sl4 = pk_t[psl_, :].rearrange(
                            "p (r f) -> p r f", r=4)[:, :, 32:256]
                        nc.vector.tensor_tensor(sl4, sl4, cxb, OP.mult)

                # ---- solve (bf16) ----
                def bc4(t):
                    return t[:].rearrange("p (o f) -> p o f", o=1) \
                               .broadcast_to([128, 4, MID])

                def r4(t):
                    return t[:].rearrange("p (r f) -> p r f", r=4)

                def r4s(ap):
                    return ap.rearrange("p (r f) -> p r f", r=4)

                def tmp(tag="t1"):
                    return pt.tile([128, MID], BF16, tag=tag, name=tag)

                def tmp4():
                    return p4.tile([128, 4 * MID], BF16, tag="t4", name="t4")

                cov = []
                for c in range(3):
                    mm = tmp4()
                    nc.vector.tensor_tensor(r4(mm), bc4(mu[c]), r4(mup), OP.mult)
                    cv = pA.tile([128, 4 * MID], BF16, tag=f"cov{c}",
                                 name=f"cov{c}")
                    nc.vector.tensor_tensor(cv[:], mip[c][:], mm[:], OP.subtract)
                    cov.append(cv)

                A = {}
                for (c, c2) in [(0, 1), (0, 2), (1, 2)]:
                    t1 = tmp()
                    nc.vector.tensor_tensor(t1[:], mu[c][:], mu[c2][:], OP.mult)
                    Ao = pA.tile([128, MID], BF16, tag=f"A{c}{c2}",
                                 name=f"A{c}{c2}")
                    nc.vector.tensor_tensor(Ao[:], mii[(c, c2)][:], t1[:],
                                            OP.subtract)
                    A[(c, c2)] = Ao
                for c in range(3):
                    sq = tmp()
                    nc.scalar.activation(sq[:], mu[c][:], AF.Square)
                    Ao = pA.tile([128, MID], BF16, tag=f"A{c}{c}",
                                 name=f"A{c}{c}")
                    nc.vector.scalar_tensor_tensor(
                        Ao[:], mii[(c, c)][:], float(EPS), sq[:],
                        OP.add, OP.subtract)
                    A[(c, c)] = Ao

                def cof(x, y, u, v=None):
                    m1 = tmp()
                    nc.vector.tensor_tensor(m1[:], A[x][:], A[y][:], OP.mult)
                    m2 = tmp()
                    if v is None:
                        nc.scalar.activation(m2[:], A[u][:], AF.Square)
                    else:
                        nc.vector.tensor_tensor(m2[:], A[u][:], A[v][:], OP.mult)
                    o = pt.tile([128, MID], BF16, tag="cof", name="cof")
                    nc.vector.tensor_tensor(o[:], m1[:], m2[:], OP.subtract)
                    return o

                c00 = cof((1, 1), (2, 2), (1, 2))
                c01 = cof((0, 2), (1, 2), (0, 1), (2, 2))
                c02 = cof((0, 1), (1, 2), (0, 2), (1, 1))
                c11 = cof((0, 0), (2, 2), (0, 2))
                c12 = cof((0, 1), (0, 2), (0, 0), (1, 2))
                c22 = cof((0, 0), (1, 1), (0, 1))

                d1 = tmp()
                nc.vector.tensor_tensor(d1[:], A[(0, 0)][:], c00[:], OP.mult)
                d2 = tmp()
                nc.vector.tensor_tensor(d2[:], A[(0, 1)][:], c01[:], OP.mult)
                d3 = tmp()
                nc.vector.tensor_tensor(d3[:], d1[:], d2[:], OP.add)
                d4 = tmp()
                nc.vector.tensor_tensor(d4[:], A[(0, 2)][:], c02[:], OP.mult)
                det = pA.tile([128, MID], F32, tag="det", name="det")
                nc.vector.tensor_tensor(det[:], d3[:], d4[:], OP.add)
                rdet = pA.tile([128, MID], F32, tag="rdet", name="rdet")
                nc.vector.reciprocal_approx_fast(rdet[:], det[:])

                adj = {}
                for nm, ct in [((0, 0), c00), ((0, 1), c01), ((0, 2), c02),
                               ((1, 1), c11), ((1, 2), c12), ((2, 2), c22)]:
                    o = pt.tile([128, MID], BF16, tag="adj", name="adj")
                    nc.vector.tensor_tensor(o[:], ct[:], rdet[:], OP.mult)
                    adj[nm] = o

                def adjget(c, j):
                    return adj[(min(c, j), max(c, j))]

                ab_all = pk.tile([128, NAB * MID], BF16, tag="aball",
                                 name="aball")
                for c in range(3):
                    asl = ab_all[:, 1152 * c:1152 * (c + 1)]
                    m1 = tmp4()
                    nc.vector.tensor_tensor(r4(m1), bc4(adjget(c, 0)),
                                            r4(cov[0]), OP.mult)
                    m2 = tmp4()
                    nc.vector.tensor_tensor(r4(m2), bc4(adjget(c, 1)),
                                            r4(cov[1]), OP.mult)
                    s1 = tmp4()
                    nc.vector.tensor_tensor(s1[:], m1[:], m2[:], OP.add)
                    m3 = tmp4()
                    nc.vector.tensor_tensor(r4(m3), bc4(adjget(c, 2)),
                                            r4(cov[2]), OP.mult)
                    nc.vector.tensor_tensor(asl, s1[:], m3[:], OP.add)

                m1 = tmp4()
                nc.vector.tensor_tensor(
                    r4(m1), bc4(mu[0]),
                    r4s(ab_all[:, 0:1152]), OP.mult)
                m2 = tmp4()
                nc.vector.tensor_tensor(
                    r4(m2), bc4(mu[1]),
                    r4s(ab_all[:, 1152:2304]), OP.mult)
                s1 = tmp4()
                nc.vector.tensor_tensor(s1[:], m1[:], m2[:], OP.add)
                m3 = tmp4()
                nc.vector.tensor_tensor(
                    r4(m3), bc4(mu[2]),
                    r4s(ab_all[:, 2304:3456]), OP.mult)
                s2 = tmp4()
                nc.vector.tensor_tensor(s2[:], s1[:], m3[:], OP.add)
                nc.vector.tensor_tensor(ab_all[:, 3456:4608], mup[:], s2[:],
                                        OP.subtract)

                nc.gpsimd.dma_start(
                    abdr[:, 128 * i:128 * (i + 1), :].rearrange("q p f -> p q f"),
                    ab_all[:].rearrange("p (q f) -> p q f", q=NAB))

                entries[i] = ab_all
                if i >= 1:
                    emit_r2cx(i - 1)
            emit_r2cx(7)

        # ---------------- stage C: R2 conv_y + q ----------------
        chunks = [(0, 96, 0, "B96b", "D96"), (1, 96, 96, "B96b", "D96"),
                  (2, 64, 160, "B96c2", "D64")]
        with tc.tile_pool(name="stC", bufs=4) as pc, \
             tc.tile_pool(name="stCq", bufs=2) as pq, \
             tc.tile_pool(name="stCm", bufs=16) as pm, \
             tc.tile_pool(name="stCc", bufs=1) as pcc, \
             tc.tile_pool(name="psumC", bufs=4, space="PSUM") as psumC:
            c2xt = pcc.tile([96, 32 * 3], F32, tag="c2xt", name="c2xt")
            for t in range(3):
                m = 96 if t < 2 else 64
                nc.scalar.dma_start(c2xt[:m, 32 * t:32 * (t + 1)],
                                    c2x[96 * t:96 * t + m, :])
            c2yt = pcc.tile([96, 992], F32, tag="c2yt", name="c2yt")
            nc.scalar.dma_start(c2yt[0:32, :], c2y[0:32, :])
            nc.scalar.dma_start(c2yt[32:64, :], c2y[32:64, :])
            for (t, m, koff, bg, bd) in chunks:
                j0 = 96 * t
                inat = []
                for c in range(3):
                    it = pc.tile([96, W], BF16, tag=f"inat{c}", name=f"inat{c}")
                    nc.scalar.dma_start(
                        it[:m, :], nat[c, 2 * R + j0:2 * R + j0 + m, R:R + W])
                    inat.append(it)
                cxadr_flat = cxadr.rearrange("q m w -> (q m) w")
                abdr_flat = abdr.rearrange("q m w -> (q m) w")
                cxns, abns = [], []
                for g in range(4):
                    gsl = slice(4 * g * W, 4 * (g + 1) * W)
                    cxn4 = pc.tile([128, 4 * W], BF16, tag="cxn", name="cxn")
                    nc.sync.dma_start_transpose(
                        cxn4[:], cxadr_flat[gsl, koff:koff + 128])
                    cxns.append(cxn4)
                    abn4 = pc.tile([128, 4 * W], BF16, tag="abn", name="abn")
                    nc.sync.dma_start_transpose(
                        abn4[:], abdr_flat[gsl, koff:koff + 128])
                    abns.append(abn4)
                means = []
                for p in range(NAB):
                    cxn = cxns[p // 4][:, W * (p % 4):W * (p % 4 + 1)]
                    abn = abns[p // 4][:, W * (p % 4):W * (p % 4 + 1)]
                    mean = pm.tile([96, W], F32, tag="mean", name="mean")
                    for s in range(2):
                        sl = slice(512 * s, 512 * (s + 1))
                        ps = psumC.tile([96, 512], F32, tag="psC", name="psC")
                        nc.tensor.matmul(ps[:m, :], Bt[bg][:, :m],
                                         cxn[:, sl], start=True, stop=False)
                        nc.tensor.matmul(ps[:m, :], Bt[bd][:, :m],
                                         abn[:, sl], start=False, stop=True)
                        nc.scalar.copy(mean[:m, sl], ps[:m, :])
                    # strip fixups: x-strips FD {0:16, 1008:1024}
                    nc.vector.tensor_tensor(
                        mean[:m, 0:16], mean[:m, 0:16],
                        c2xt[:m, 32 * t:32 * t + 16], OP.mult)
                    nc.vector.tensor_tensor(
                        mean[:m, 1008:1024], mean[:m, 1008:1024],
                        c2xt[:m, 32 * t + 16:32 * t + 32], OP.mult)
                    if t == 0:
                        nc.vector.tensor_tensor(mean[0:32, 16:1008],
                                                mean[0:32, 16:1008],
                                                c2yt[0:32, :], OP.mult)
                    if t == 2:
                        nc.vector.tensor_tensor(mean[32:64, 16:1008],
                                                mean[32:64, 16:1008],
                                                c2yt[32:64, :], OP.mult)
                    means.append(mean)
                for k in range(4):
                    m1 = pq.tile([96, W], F32, tag="qm1", name="qm1")
                    nc.vector.tensor_tensor(m1[:m, :], means[k][:m, :],
                                            inat[0][:m, :], OP.mult)
                    m2 = pq.tile([96, W], F32, tag="qm2", name="qm2")
                    nc.vector.tensor_tensor(m2[:m, :], means[4 + k][:m, :],
                                            inat[1][:m, :], OP.mult)
                    s1 = pq.tile([96, W], F32, tag="qs1", name="qs1")
                    nc.vector.tensor_tensor(s1[:m, :], m1[:m, :], m2[:m, :],
                                            OP.add)
                    m3 = pq.tile([96, W], F32, tag="qm3", name="qm3")
                    nc.vector.tensor_tensor(m3[:m, :], means[8 + k][:m, :],
                                            inat[2][:m, :], OP.mult)
                    s2 = pq.tile([96, W], F32, tag="qs2", name="qs2")
                    nc.vector.tensor_tensor(s2[:m, :], s1[:m, :], m3[:m, :],
                                            OP.add)
                    qo = pq.tile([96, W], F32, tag="qo", name="qo")
                    nc.vector.scalar_tensor_tensor(
                        qo[:m, :], s2[:m, :], 0.5, means[12 + k][:m, :],
                        OP.add, OP.add)
                    nc.gpsimd.dma_start(qdr[k, j0:j0 + m, :], qo[:m, :])

    nc.compile()
    return nc


_NC_CACHE = None


def _get_nc():
    global _NC_CACHE
    if _NC_CACHE is None:
        _NC_CACHE = _build()
    return _NC_CACHE


# ----------------------------------------------------------------- host side

def _host_prep(I, p):
    If = I.astype(np.float64) - 0.5
    pf = p.astype(np.float64) - 0.5
    band_arrs = {k: v.astype(bf16) for k, v in BANDS.items()}
    strip_cache = {}
    maps = []
    for core in range(N_CORES):
        b, i = divmod(core, 4)
        r0 = i * ROWS
        planes = [If[b, :, :, c] for c in range(C)]
        planes += [pf[b, :, :, k] for k in range(K)]
        for c in range(C):
            for k in range(K):
                planes.append(If[b, :, :, c] * pf[b, :, :, k])
        for c in range(C):
            for c2 in range(c, C):
                planes.append(If[b, :, :, c] * If[b, :, :, c2])
        planes = np.stack(planes)

        nat = np.zeros((NPL, EXT, WE), np.float64)
        ylo = r0 - 2 * R
        sy0, sy1 = max(0, ylo), min(H, r0 + ROWS + 2 * R)
        nat[:, sy0 - ylo:sy1 - ylo, R:R + W] = planes[:, sy0:sy1, :]
        nat = nat.astype(bf16)
        xT = np.ascontiguousarray(nat[:, R:R + MID, R:R + W].transpose(0, 2, 1))

        if i not in strip_cache:
            S = _S2D
            ymid0 = r0 - R
            yy = np.arange(ymid0, ymid0 + MID)
            cmid = np.zeros((MID, W))
            valid = (yy >= 0) & (yy < H)
            cmid[valid] = S / _NFULL[yy[valid]]
            # y-strip planes (c on mid idx [0:32) and [256:288)), [W, 64]
            cy1 = np.concatenate([cmid[0:32, :].T, cmid[256:288, :].T],
                                 axis=1).astype(np.float32)
            # x-strips for chunks 0 and 7: partitions [0:16)/[112:128),
            # FD (mid idx) [32:256)
            cx1a = np.ones((32, 224), np.float32)
            cx1a[0:16] = cmid[32:256, 0:16].T
            cx1b = np.ones((32, 224), np.float32)
            cx1b[16:32] = cmid[32:256, 1008:1024].T
            # stage C: out rows r0..r0+256
            c2full = S / _NFULL[r0:r0 + ROWS]
            c2x = np.concatenate([c2full[:, 0:16], c2full[:, 1008:1024]],
                                 axis=1).astype(np.float32)
            c2y = np.ones((64, 992), np.float32)
            c2y[0:16] = c2full[0:16, 16:1008]
            c2y[48:64] = c2full[240:256, 16:1008]
            strip_cache[i] = (cy1, cx1a, cx1b, c2x, c2y)
        cy1, cx1a, cx1b, c2x, c2y = strip_cache[i]

        m = dict(nat=nat, xT=xT, cy1=cy1, cx1a=cx1a, cx1b=cx1b,
                 c2x=c2x, c2y=c2y)
        m.update(band_arrs)
        maps.append(m)
    return maps


def kernel(I, p):
    I = np.asarray(I)
    p = np.asarray(p)
    nc = _get_nc()
    in_maps = _host_prep(I, p)
    res = run_bass_kernel_spmd(nc, in_maps, core_ids=list(range(N_CORES)))
    out = np.zeros((B, H, W, K), np.float32)
    for core in range(N_CORES):
        b, i = divmod(core, 4)
        r0 = i * ROWS
        q = res.results[core]["qdr"]  # [K, ROWS, W]
        out[b, r0:r0 + ROWS] = q.transpose(1, 2, 0)
    return out
